# revision 38
# baseline (speedup 1.0000x reference)
"""AttentionBlock (GroupNorm32 + QKV 8-head attention + proj + residual) on 8 TRN2 NeuronCores.

Sharding: pure data-parallel over batch B=8 — one batch element per core.

v5 design:
  - Scalar-engine exp of the logits is the binding resource (~66us busy).  The
    kernel is 4 sweeps (head-group g x t-half n) of 16 (pair, sm) units, each
    unit = 2 quadrant QK matmuls + one [128,1024] exp.  The exp stream must be
    gap-free: ONLY QK/exp use the "psw" psum tag (bufs=2); every other matmul
    (v, spare q/k c-tiles, AV bursts, recip broadcast, proj) runs as a
    self-contained chunk on the "acc" tag so the psw rotation never blocks.
  - AV is deferred: per (g,n,head) a 4-instruction fp8-DoubleRow burst over the
    completed ew tile, run as fillers in the NEXT sweep (last sweep: smp0-2
    right after its last QK, smp3 on the tail).  vT carries a ones-column for
    softmax row sums; denominators via reciprocal_approx_fast + select matmul.
  - QK is bf16 (output-rate bound; fp8 DoubleRow does not raise the column
    rate).  x input is bf16 (host-cast).  v/proj are fp8 DoubleRow.
"""

import numpy as np
import ml_dtypes
from contextlib import ExitStack

import concourse.bass as bass
import concourse.tile as tile
from concourse import bacc, mybir
from concourse.bass_utils import run_bass_kernel_spmd

F32 = mybir.dt.float32
BF = mybir.dt.bfloat16
F8 = mybir.dt.float8e4
MULT = mybir.AluOpType.mult
ADD = mybir.AluOpType.add
AFT = mybir.ActivationFunctionType
DRM = mybir.MatmulPerfMode.DoubleRow

C, T, H, CH = 512, 1024, 8, 64
NJ = C // 128          # 4 c-tiles
NTM = T // 128         # 8 t-tiles (s-chunks)
EXP_SCALE = float(CH) ** -0.5
EXP_BIAS = -2.0        # keeps exp() under fp8e4 max (240); cancels in softmax

BF_NP = ml_dtypes.bfloat16
F8_NP = ml_dtypes.float8_e4m3


def build_graph(enable_asserts: bool = False):
    nc = bacc.Bacc(
        "TRN2",
        target_bir_lowering=False,
        debug=False,
        enable_asserts=enable_asserts,
    )
    x_d = nc.dram_tensor("x", [C, T], BF, kind="ExternalInput").ap()
    wq_d = nc.dram_tensor("wq", [C, C], BF, kind="ExternalInput").ap()
    wk_d = nc.dram_tensor("wk", [C, C], BF, kind="ExternalInput").ap()
    wv_d = nc.dram_tensor("wv", [C, C], F8, kind="ExternalInput").ap()
    pw_d = nc.dram_tensor("pw", [C, C], F8, kind="ExternalInput").ap()
    bq_d = nc.dram_tensor("bq", [C], F32, kind="ExternalInput").ap()
    bk_d = nc.dram_tensor("bk", [C], F32, kind="ExternalInput").ap()
    bv_d = nc.dram_tensor("bv", [C], F32, kind="ExternalInput").ap()
    pb_d = nc.dram_tensor("pb", [C], F32, kind="ExternalInput").ap()
    gns_d = nc.dram_tensor("gns", [C], F32, kind="ExternalInput").ap()
    gnb_d = nc.dram_tensor("gnb", [C], F32, kind="ExternalInput").ap()
    g8_d = nc.dram_tensor("g8", [128, 8], F32, kind="ExternalInput").ap()   # pre-scaled 1/16
    gt8_d = nc.dram_tensor("gt8", [8, 128], F32, kind="ExternalInput").ap()
    sel4_d = nc.dram_tensor("sel4", [4, 2 * 128], BF, kind="ExternalInput").ap()
    out_d = nc.dram_tensor("out", [C, T], F32, kind="ExternalOutput").ap()

    with tile.TileContext(nc) as tc, ExitStack() as ctx:
        consts = ctx.enter_context(tc.tile_pool(name="consts", bufs=1))
        bigs = ctx.enter_context(tc.tile_pool(name="bigs", bufs=1))
        ewp = ctx.enter_context(tc.tile_pool(name="ewp", bufs=2))
        work = ctx.enter_context(tc.tile_pool(name="work", bufs=2))
        outp = ctx.enter_context(tc.tile_pool(name="outp", bufs=1))
        psum = ctx.enter_context(tc.tile_pool(name="psum", bufs=1, space="PSUM"))

        def pswt(name):
            return psum.tile([128, 2, 512], F32, tag="psw", bufs=3, name=name)

        def acct(name, shape=(128, 512)):
            return psum.tile(list(shape), F32, tag="acc", bufs=2, name=name)

        # ---- persistent sbuf tensors ----
        xt = bigs.tile([128, NJ, T], BF)            # raw x (bf16), kept for residual
        xn = bigs.tile([128, NJ, T], BF)            # groupnormed x (bf16, q/k path)
        xn8 = bigs.tile([128, NJ, T], F8)           # groupnormed x (fp8, v path)
        q_sb = bigs.tile([128, NJ, T], BF)          # q rows (head-major)
        k_sb = bigs.tile([128, NJ, T], BF)          # k rows (head-major)
        vT_sb = bigs.tile([128, NTM, H, 128], F8)   # v transposed + ones col, padded
        a_un = bigs.tile([128, NJ, T], BF)          # unnormalized attention out
        a_n = bigs.tile([128, NJ, T], F8)           # normalized attention out
        osb = outp.tile([128, NJ, T], F32)          # assembled output

        # ---- input DMAs (ordered by first use; x split fine for queue spread) ----
        for j in range(NJ):
            for hf in range(2):
                nc.sync.dma_start(xt[:, j, 512 * hf:512 * (hf + 1)],
                                  x_d[j * 128:(j + 1) * 128, 512 * hf:512 * (hf + 1)])
        gns_sb = consts.tile([128, NJ], F32)
        gnb_sb = consts.tile([128, NJ], F32)
        nc.sync.dma_start(gns_sb[:], bass.AP(tensor=gns_d.tensor, offset=0, ap=[[1, 128], [128, NJ]]))
        nc.sync.dma_start(gnb_sb[:], bass.AP(tensor=gnb_d.tensor, offset=0, ap=[[1, 128], [128, NJ]]))
        g8_sb = consts.tile([128, 8], F32)
        gt8_sb = consts.tile([8, 128], F32)
        nc.sync.dma_start(g8_sb[:], g8_d[:])
        nc.sync.dma_start(gt8_sb[:], gt8_d[:])

        bq_sb = consts.tile([128, NJ], F32)
        bk_sb = consts.tile([128, NJ], F32)
        pb_sb = consts.tile([128, NJ], F32)
        for j in range(NJ):
            nc.sync.dma_start(bq_sb[:, j:j + 1], bq_d[j * 128:(j + 1) * 128])
            nc.sync.dma_start(bk_sb[:, j:j + 1], bk_d[j * 128:(j + 1) * 128])
            nc.sync.dma_start(pb_sb[:, j:j + 1], pb_d[j * 128:(j + 1) * 128])
        bv_bc = consts.tile([128, C], F32)      # v bias broadcast to all partitions
        nc.sync.dma_start(bv_bc[:], bass.AP(tensor=bv_d.tensor, offset=0, ap=[[0, 128], [1, C]]))
        sel4_sb = consts.tile([4, 2, 128], BF)
        nc.sync.dma_start(sel4_sb[:], sel4_d[:].rearrange("p (j m) -> p j m", j=2))
        # weights last: x owns the DMA queues so GroupNorm can start ASAP
        wq_sb = consts.tile([128, NJ, C], BF)
        wk_sb = consts.tile([128, NJ, C], BF)
        wv_sb = consts.tile([128, NJ, C], F8)
        pw_sb = consts.tile([128, NJ, C], F8)
        for j in range(NJ):
            nc.sync.dma_start(wk_sb[:, j, :], wk_d[j * 128:(j + 1) * 128, :])
            nc.sync.dma_start(wq_sb[:, j, :], wq_d[j * 128:(j + 1) * 128, :])
        for j in range(NJ):
            nc.sync.dma_start(wv_sb[:, j, :], wv_d[j * 128:(j + 1) * 128, :])
            nc.sync.dma_start(pw_sb[:, j, :], pw_d[j * 128:(j + 1) * 128, :])

        eps_sb = consts.tile([128, 1], F32)
        nc.vector.memset(eps_sb[:], 1e-5)
        nb_sb = consts.tile([128, 1], F32)
        nc.vector.memset(nb_sb[:], EXP_BIAS)
        # zero the vT pad columns once (Ldweights loads the full 128-col slab);
        # on GpSimd so it doesn't block the DVE preamble chain
        nc.gpsimd.memset(vT_sb[:, :, :, CH + 1:128], 0.0)

        # ---- PE warm-up: ramp the tensor engine to full pstate during DMA wait
        wz = consts.tile([128, 512], BF)
        nc.vector.memset(wz[:], 0.0)
        wrd = consts.tile([128, 4], F32)
        for r in range(2):
            wps = acct(f"wps{r}")
            for i in range(8):
                nc.tensor.matmul(wps[:], wz[:, 0:128], wz[:], start=(i == 0), stop=(i == 7))
            nc.vector.tensor_copy(wrd[:, r:r + 1], wps[:, 0:1])

        # ---- GroupNorm: per-partition stats, group-reduce via tiny f32 matmuls ----
        stats_sb = consts.tile([128, 3, NJ], F32)  # rows: mean | var | mean^2
        for j in range(NJ):
            st6 = work.tile([128, 2, 6], F32, tag="st6")
            nc.vector.bn_stats(st6[:, 0, :], xt[:, j, 0:512])
            nc.vector.bn_stats(st6[:, 1, :], xt[:, j, 512:1024])
            nc.vector.bn_aggr(stats_sb[:, 0:2, j], st6[:])
        nc.vector.tensor_mul(stats_sb[:, 2, :], stats_sb[:, 0, :], stats_sb[:, 0, :])
        ps_st = acct("ps_st", (8, 3 * NJ))
        nc.tensor.matmul(ps_st[:], g8_sb[:], stats_sb[:].rearrange("p a b -> p (a b)"),
                         start=True, stop=True)

        st_g = work.tile([8, 3 * NJ], F32, tag="stg2")
        nc.vector.tensor_copy(st_g[:], ps_st[:])
        stv = st_g[:].rearrange("p (c j) -> p c j", c=3)
        bcin = work.tile([8, 8], F32, tag="bcin")
        vv = work.tile([8, NJ], F32, tag="vv")
        nc.vector.tensor_add(vv[:], stv[:, 1, :], stv[:, 2, :])
        m2 = work.tile([8, NJ], F32, tag="m2")
        nc.vector.tensor_mul(m2[:], stv[:, 0, :], stv[:, 0, :])
        nc.vector.tensor_sub(vv[:], vv[:], m2[:])
        nc.vector.tensor_copy(bcin[:, 0:4], stv[:, 0, :])
        nc.scalar.activation(vv[:], vv[:], AFT.Sqrt, bias=eps_sb[0:8, :], scale=1.0)
        # re-warm the Exp table right after Sqrt so the stream isn't table-delayed
        warm = work.tile([1, 1], BF, tag="warm", bufs=1)
        nc.scalar.activation(warm[:], eps_sb[0:1, :], AFT.Exp, bias=eps_sb[0:1, :], scale=1.0)
        nc.vector.reciprocal(bcin[:, 4:8], vv[:])
        ps_pp = acct("ps_pp", (128, 8))
        nc.tensor.matmul(ps_pp[:], gt8_sb[:], bcin[:], start=True, stop=True)

        ab = work.tile([128, 2 * NJ], F32, tag="ab")   # scale | shift per c-tile
        t4 = work.tile([128, NJ], F32, tag="t4")
        nc.vector.tensor_mul(ab[:, 0:NJ], ps_pp[:, 4:8], gns_sb[:])
        nc.vector.tensor_mul(t4[:], ps_pp[:, 0:4], ab[:, 0:NJ])
        nc.vector.tensor_sub(ab[:, NJ:2 * NJ], gnb_sb[:], t4[:])
        for j in range(NJ):
            nc.vector.tensor_scalar(xn[:, j, :], xt[:, j, :],
                                    ab[:, j:j + 1], ab[:, NJ + j:NJ + j + 1],
                                    op0=MULT, op1=ADD)

        # ---- filler chunk makers (all self-contained on the acc tag) ----
        kq_state = {}

        def kq_sub(m, part, n, half):
            # half a k/q [128,512] psum group (2 of 4 j chunks); the group
            # stays open across the two consecutive filler units
            def fn():
                w_sb = wk_sb if part == "k" else wq_sb
                dst = k_sb if part == "k" else q_sb
                b_sb = bk_sb if part == "k" else bq_sb
                if half == 0:
                    kq_state[(m, part, n)] = acct(f"ps{part}{m}{n}")
                ps = kq_state[(m, part, n)]
                for j in (0, 1) if half == 0 else (2, 3):
                    nc.tensor.matmul(ps[:],
                                     w_sb[:, j, 128 * m:128 * (m + 1)],
                                     xn[:, j, 512 * n:512 * (n + 1)],
                                     start=(j == 0), stop=(j == NJ - 1))
                if half == 1:
                    nc.vector.tensor_scalar(dst[:, m, 512 * n:512 * (n + 1)], ps[:],
                                            b_sb[:, m:m + 1], None, op0=ADD)
            return fn

        def kq_chunk(m, part, n):
            a, b = kq_sub(m, part, n, 0), kq_sub(m, part, n, 1)

            def fn():
                a(); b()
            return fn

        def v_single(tm):
            def fn():
                psv = acct(f"psv{tm}")
                for jp in range(2):
                    nc.tensor.matmul(psv[:],
                                     xn8[:, 2 * jp:2 * jp + 2, 128 * tm:128 * (tm + 1)],
                                     wv_sb[:, 2 * jp:2 * jp + 2, :],
                                     start=(jp == 0), stop=(jp == 1), perf_mode=DRM)
                nc.vector.tensor_add(vT_sb[:, tm, :, 0:CH],
                                     psv[:].rearrange("p (h c) -> p h c", h=H),
                                     bv_bc[:].rearrange("p (h c) -> p h c", h=H))
                nc.vector.memset(vT_sb[:, tm, :, CH:CH + 1], 1.0)
            return fn

        # PE pstate heater: dummy back-to-back matmuls; PE downclocks ~2x after
        # any idle gap, so the schedule keeps it saturated with filler heat
        hrd = consts.tile([128, 4], F32)
        heat_ctr = [0]

        def heater(kk):
            def fn():
                i = heat_ctr[0]
                heat_ctr[0] += 1
                hp = acct(f"heat{i}")
                for _ in range(kk):
                    nc.tensor.matmul(hp[:], wz[:, 0:128], wz[:], start=True, stop=True)
                nc.vector.tensor_copy(hrd[:, i % 4:i % 4 + 1], hp[:, 0:1])
            return fn

        def proj_half(ms, n):
            # proj output halves for c-tiles ms at t-half n (fp8 DoubleRow)
            def fn():
                for m in (ms if isinstance(ms, tuple) else (ms,)):
                    psp = acct(f"psp{m}{n}")
                    for jp in range(2):
                        nc.tensor.matmul(psp[:],
                                         pw_sb[:, 2 * jp:2 * jp + 2, 128 * m:128 * (m + 1)],
                                         a_n[:, 2 * jp:2 * jp + 2, 512 * n:512 * (n + 1)],
                                         start=(jp == 0), stop=(jp == 1), perf_mode=DRM)
                    nc.vector.scalar_tensor_tensor(osb[:, m, 512 * n:512 * (n + 1)],
                                                   psp[:], pb_sb[:, m:m + 1],
                                                   xt[:, m, 512 * n:512 * (n + 1)],
                                                   op0=ADD, op1=ADD)
                    nc.sync.dma_start(out_d[128 * m:(m + 1) * 128, 512 * n:512 * (n + 1)],
                                      osb[:, m, 512 * n:512 * (n + 1)])
            return fn

        # ---- attention sweeps: 16 (pair-half u, s-chunk sm) units ----
        def attention_sweep(g, n, unit_order, fillers, sw=None):
            ew = ewp.tile([128, NTM, 4, 512], F8, tag="ew", name=f"ew{g}{n}")
            sw = {} if sw is None else sw
            sw.update(g=g, n=n, ew=ew)
            sw["stg"] = work.tile([65, 4, 512], F32, tag="stg", name=f"stg{g}{n}")
            for idx, (u, sm) in enumerate(unit_order):
                p = 2 * g + u
                psw = pswt(f"psw{g}{n}{sm}{u}")
                for uu in range(2):
                    nc.tensor.matmul(psw[:, uu, :],
                                     k_sb[64 * uu:64 * (uu + 1), p, 128 * sm:128 * (sm + 1)],
                                     q_sb[64 * uu:64 * (uu + 1), p, 512 * n:512 * (n + 1)],
                                     start=True, stop=True, tile_position=(64 * uu, 0))
                nc.scalar.activation(ew[:, sm, 2 * u:2 * u + 2, :], psw[:],
                                     AFT.Exp, bias=nb_sb[:], scale=EXP_SCALE)
                for f in fillers.get(idx, []):
                    f()
            return sw

        # AV burst for one head: fp8 DoubleRow over sm-pairs of the DONE ew tile.
        # Split into sub-bursts; the psum group stays open between them (no
        # other acc-tag allocation may occur in between).
        def av_sub(sw, hh, smps, begin, finish, add_in):
            def fn():
                g, n, ew, stg = sw["g"], sw["n"], sw["ew"], sw["stg"]
                h = 4 * g + hh
                if begin:
                    sw[f"psa{hh}"] = acct(f"psa{g}{n}{hh}{smps[0]}")
                psa = sw[f"psa{hh}"]
                for i, smp in enumerate(smps):
                    nc.tensor.matmul(psa[:],
                                     vT_sb[:, 2 * smp:2 * smp + 2, h, :],
                                     ew[:, 2 * smp:2 * smp + 2, hh, :],
                                     start=(begin and i == 0),
                                     stop=(finish and i == len(smps) - 1),
                                     perf_mode=DRM)
                if finish:
                    au = a_un[64 * (h % 2):64 * (h % 2) + 64, h // 2,
                              512 * n:512 * (n + 1)]
                    if add_in:
                        nc.vector.tensor_add(stg[64:65, hh, :], psa[CH:CH + 1, :],
                                             stg[64:65, hh, :])
                        nc.vector.tensor_add(au, psa[0:CH, :], au)
                    else:
                        nc.vector.tensor_copy(stg[64:65, hh, :], psa[CH:CH + 1, :])
                        nc.vector.tensor_copy(au, psa[0:CH, :])
            return fn

        def av_burst(sw, hh, smps, add_in):
            def fn():
                av_sub(sw, hh, smps, True, True, add_in)()
            return fn

        def ep_rs(sw):
            rs4 = work.tile([4, 512], F32, tag="rs4")
            rc4 = work.tile([4, 512], F32, tag="rc4")
            sw["rc4"] = rc4

            def fn():
                nc.sync.dma_start(rs4[:], sw["stg"][64:65, :, :])
                nc.vector.reciprocal_approx_fast(rc4[:], rs4[:])
            return fn

        def ep_norm_j(sw, jj):
            def fn():
                g, n = sw["g"], sw["n"]
                if jj == 0:
                    rcb = work.tile([4, 512], BF, tag="rcb", name=f"rcb{g}{n}")
                    nc.vector.tensor_copy(rcb[:], sw["rc4"][:])
                    sw["rcb"] = rcb
                j = 2 * g + jj
                psb = acct(f"psb{g}{n}{jj}")
                nc.tensor.matmul(psb[:], sel4_sb[:, jj, :], sw["rcb"][:],
                                 start=True, stop=True)
                nc.vector.tensor_mul(a_n[:, j, 512 * n:512 * (n + 1)],
                                     a_un[:, j, 512 * n:512 * (n + 1)], psb[:])
            return fn

        def ep_norm(sw):
            def fn():
                ep_norm_j(sw, 0)()
                ep_norm_j(sw, 1)()
            return fn

        PMAJ = [(u, sm) for u in range(2) for sm in range(NTM)]
        SMAJ = [(u, sm) for sm in range(NTM) for u in range(2)]
        ALLSMP = list(range(NTM // 2))

        # preamble: k0 fully + q0 first half (all sweep00 needs); the rest rides
        # the sweep as fillers
        kq_chunk(0, "k", 0)()
        kq_chunk(0, "k", 1)()
        kq_chunk(0, "q", 0)()

        def xn8_cast():
            def fn():
                for j in range(NJ):
                    nc.vector.tensor_copy(xn8[:, j, :], xn[:, j, :])
            return fn

        def F(*fs):
            return list(fs)

        def xn8_half(j0):
            def fn():
                nc.vector.tensor_copy(xn8[:, j0, :], xn[:, j0, :])
                nc.vector.tensor_copy(xn8[:, j0 + 1, :], xn[:, j0 + 1, :])
            return fn

        sw00 = attention_sweep(0, 0, PMAJ, {
            0: F(kq_sub(1, "k", 0, 0)), 1: F(kq_sub(1, "k", 0, 1)),
            2: F(kq_sub(1, "q", 0, 0)), 3: F(kq_sub(1, "q", 0, 1)),
            4: F(kq_sub(1, "k", 1, 0)), 5: F(kq_sub(1, "k", 1, 1)),
            6: F(kq_sub(0, "q", 1, 0)), 7: F(kq_sub(0, "q", 1, 1)),
            8: F(kq_sub(1, "q", 1, 0)), 9: F(kq_sub(1, "q", 1, 1)),
            10: F(xn8_half(0)), 11: F(xn8_half(2)),
            12: F(v_single(0)), 13: F(v_single(1)),
            14: F(v_single(2)), 15: F(v_single(3))})
        sw01 = attention_sweep(0, 1, PMAJ, {
            0: F(v_single(4)), 1: F(v_single(5)),
            2: F(v_single(6)), 3: F(v_single(7)),
            4: F(av_sub(sw00, 0, [0, 1], True, False, False)),
            5: F(av_sub(sw00, 0, [2, 3], False, True, False)),
            6: F(av_sub(sw00, 1, [0, 1], True, False, False)),
            7: F(av_sub(sw00, 1, [2, 3], False, True, False)),
            8: F(av_sub(sw00, 2, [0, 1], True, False, False)),
            9: F(av_sub(sw00, 2, [2, 3], False, True, False)),
            10: F(av_sub(sw00, 3, [0, 1], True, False, False)),
            11: F(av_sub(sw00, 3, [2, 3], False, True, False)),
            12: F(ep_rs(sw00), ep_norm_j(sw00, 0), ep_norm_j(sw00, 1)),
            13: F(kq_chunk(2, "k", 0)),
            14: F(kq_chunk(2, "k", 1)), 15: F(kq_chunk(2, "q", 0))})
        sw10 = attention_sweep(1, 0, PMAJ, {
            0: F(kq_chunk(3, "k", 0)), 1: F(kq_chunk(3, "k", 1)),
            2: F(kq_chunk(3, "q", 0)),
            3: F(av_sub(sw01, 0, [0, 1], True, False, False)),
            4: F(av_sub(sw01, 0, [2, 3], False, True, False)),
            5: F(av_sub(sw01, 1, [0, 1], True, False, False)),
            6: F(av_sub(sw01, 1, [2, 3], False, True, False)),
            7: F(av_sub(sw01, 2, [0, 1], True, False, False)),
            8: F(av_sub(sw01, 2, [2, 3], False, True, False)),
            9: F(av_sub(sw01, 3, [0, 1], True, False, False)),
            10: F(av_sub(sw01, 3, [2, 3], False, True, False)),
            11: F(ep_rs(sw01)),
            12: F(ep_norm_j(sw01, 0)), 13: F(ep_norm_j(sw01, 1)),
            14: F(kq_chunk(2, "q", 1)), 15: F(kq_chunk(3, "q", 1))})
        sw11z = {}
        sw11 = attention_sweep(1, 1, SMAJ, {
            0: F(av_sub(sw10, 0, [0, 1], True, False, False)),
            1: F(av_sub(sw10, 0, [2, 3], False, True, False)),
            2: F(av_sub(sw10, 1, [0, 1], True, False, False)),
            3: F(av_sub(sw10, 1, [2, 3], False, True, False)),
            4: F(av_sub(sw10, 2, [0, 1], True, False, False)),
            5: F(av_sub(sw10, 2, [2, 3], False, True, False)),
            6: F(av_sub(sw10, 3, [0, 1], True, False, False)),
            7: F(av_sub(sw10, 3, [2, 3], False, True, False)),
            8: F(ep_rs(sw10)),
            9: F(ep_norm_j(sw10, 0)), 10: F(ep_norm_j(sw10, 1)),
            11: F(av_sub(sw11z, 0, [0, 1, 2], True, True, False)),
            12: F(av_sub(sw11z, 1, [0, 1, 2], True, True, False), proj_half(0, 0)),
            13: F(av_sub(sw11z, 2, [0, 1, 2], True, True, False), proj_half(1, 0)),
            14: F(av_sub(sw11z, 3, [0, 1, 2], True, True, False), proj_half(2, 0)),
            15: F(proj_half(3, 0))}, sw=sw11z)
        sw11 = sw11z

        # tail: B-groups (smp3) pairwise so only 2 psa tiles are ever live
        # (acc bufs=2), row-sum adds ahead of the a_un adds, proj-n0 matmuls
        # interleaved to keep the PE fed while the DVE chain drains.
        stg11, ew11 = sw11["stg"], sw11["ew"]

        def tail_b_mm(hh):
            sw11[f"psb{hh}"] = acct(f"psat{hh}")
            nc.tensor.matmul(sw11[f"psb{hh}"][:], vT_sb[:, 6:8, 4 + hh, :],
                             ew11[:, 6:8, hh, :], start=True, stop=True,
                             perf_mode=DRM)
            nc.vector.tensor_add(stg11[64:65, hh, :],
                                 sw11[f"psb{hh}"][CH:CH + 1, :], stg11[64:65, hh, :])

        def tail_b_au(hh):
            h = 4 + hh
            au = a_un[64 * (h % 2):64 * (h % 2) + 64, h // 2, 512:1024]
            nc.vector.tensor_add(au, sw11[f"psb{hh}"][0:CH, :], au)

        tail_b_mm(0)
        tail_b_mm(1)
        tail_b_au(0)
        tail_b_mm(2)
        tail_b_au(1)
        tail_b_mm(3)
        ep_rs(sw11)()
        tail_b_au(2)
        tail_b_au(3)
        ep_norm(sw11)()
        proj_half((0, 1), 1)()
        proj_half((2, 3), 1)()

    nc.compile()
    return nc


_NC_CACHE = {}


def get_nc():
    if "nc" not in _NC_CACHE:
        _NC_CACHE["nc"] = build_graph()
    return _NC_CACHE["nc"]


def make_in_maps(x, norm_scale, norm_bias, qkv_w, qkv_b, proj_w, proj_b):
    x = np.asarray(x, dtype=np.float32)
    B = x.shape[0]
    qr = np.asarray(qkv_w, np.float32).reshape(H, 3, CH, C)
    wq = np.ascontiguousarray(qr[:, 0].reshape(C, C).T).astype(BF_NP)
    wk = np.ascontiguousarray(qr[:, 1].reshape(C, C).T).astype(BF_NP)
    wv = np.ascontiguousarray(qr[:, 2].reshape(C, C).T).astype(F8_NP)
    br = np.asarray(qkv_b, np.float32).reshape(H, 3, CH)
    bq = np.ascontiguousarray(br[:, 0].reshape(C))
    bk = np.ascontiguousarray(br[:, 1].reshape(C))
    bv = np.ascontiguousarray(br[:, 2].reshape(C))
    pw = np.ascontiguousarray(np.asarray(proj_w, np.float32).T).astype(F8_NP)
    pb = np.ascontiguousarray(np.asarray(proj_b, np.float32))
    g8 = np.zeros((128, 8), np.float32)
    g8[np.arange(128), np.arange(128) // 16] = 1.0 / 16.0
    gt8 = np.ascontiguousarray((g8 != 0).astype(np.float32).T)
    sel4 = np.zeros((4, 2, 128), np.float32)
    for jj in range(2):
        p = np.arange(128)
        sel4[2 * jj + p // 64, jj, p] = 1.0
    sel4 = np.ascontiguousarray(sel4.reshape(4, 256)).astype(BF_NP)
    shared = dict(wq=wq, wk=wk, wv=wv, pw=pw, bq=bq, bk=bk, bv=bv, pb=pb,
                  sel4=sel4,
                  gns=np.ascontiguousarray(np.asarray(norm_scale, np.float32)),
                  gnb=np.ascontiguousarray(np.asarray(norm_bias, np.float32)),
                  g8=g8, gt8=gt8)
    in_maps = []
    for i in range(B):
        m = dict(shared)
        m["x"] = np.ascontiguousarray(x[i].reshape(C, T)).astype(BF_NP)
        in_maps.append(m)
    return in_maps


def kernel(x, norm_scale, norm_bias, qkv_w, qkv_b, proj_w, proj_b):
    x = np.asarray(x, dtype=np.float32)
    B, Cc, Hh, Ww = x.shape
    nc = get_nc()
    in_maps = make_in_maps(x, norm_scale, norm_bias, qkv_w, qkv_b, proj_w, proj_b)
    res = run_bass_kernel_spmd(nc, in_maps, core_ids=list(range(B)))
    out = np.stack([res.results[i]["out"] for i in range(B)])
    return out.reshape(B, Cc, Hh, Ww).astype(np.float32)
